# revision 23
# baseline (speedup 1.0000x reference)
"""Trainium2 Bass kernel for nn_DirDist_P2P (retrieval_knn).

Computes the UDF direction-distance metric between two point clouds:
Q = 2048*10 jittered tgt queries + 2048 src queries; K=5 NN in each cloud
with inverse-distance weighting; weighted scalar sum.

Strategy (8 cores, seed-parallel):
  - each core owns 256 tgt seeds (-> 2560 jittered queries) and 256 src queries;
    both full clouds (2048 pts each) are replicated per core.
  - stage 1: -d^2(seed, cloud) via one fp32 matmul in homogeneous coords;
    DVE max/max_index per half -> 16 candidate NNs per tgt seed
    (for src queries: exact sorted top-8 of the full row).
  - gather candidate coords via per-column indirect DMA (128 rows/call).
  - stage 2 (tgt jitter queries): exact d^2(query, candidate) in seed-major
    layout [128 seeds, 10 jitters, 16 cands]; per-jitter top-8 max; threshold
    mask -> inverse-distance weights -> weighted centroid -> udf/grad.
  - src queries: top-5 straight from the sorted stage-1 top-8.
  - cross-cloud combine, per-partition accumulate; host sums the 8 partials.
"""
import os
import sys

sys.path.insert(0, "/opt/trn_rl_repo")

import numpy as np

CH = 5                      # gathered candidates per half (2*CH total per seed)
P = 128
N = 2048
UP = 10
NCORE = 8
SEEDS = N // NCORE          # 256 per core
NT = SEEDS // P             # 2 seed tiles per core
STD = 0.05
EPS_D = 1e-8
EPS_N = 1e-10
BETA = 3.0
NQ = N * UP + N             # 22528 total queries

_PROG = None
LAST_EXEC_NS = None


def _build():
    import concourse.bass as bass
    import concourse.bass_isa as bass_isa
    import concourse.bacc as bacc
    import concourse.mybir as mybir
    from concourse.tile import TileContext

    F32 = mybir.dt.float32
    U32 = mybir.dt.uint32
    AF = mybir.ActivationFunctionType
    ALU = mybir.AluOpType
    AX = mybir.AxisListType

    nc = bacc.Bacc("TRN2", target_bir_lowering=False, debug=False, num_devices=NCORE)

    pts_d = {
        "t": nc.dram_tensor("tgt_pts", [N, 3], F32, kind="ExternalInput").ap(),
        "s": nc.dram_tensor("src_pts", [N, 3], F32, kind="ExternalInput").ap(),
    }
    xT_d = {
        "t": nc.dram_tensor("tgtT", [3, N], F32, kind="ExternalInput").ap(),
        "s": nc.dram_tensor("srcT", [3, N], F32, kind="ExternalInput").ap(),
    }
    seedT_d = nc.dram_tensor("seedT", [3, 2 * SEEDS], F32, kind="ExternalInput").ap()
    my_tgt_d = nc.dram_tensor("my_tgt", [SEEDS, 3], F32, kind="ExternalInput").ap()
    my_src_d = nc.dram_tensor("my_src", [SEEDS, 3], F32, kind="ExternalInput").ap()
    my_noise_d = nc.dram_tensor("my_noise", [SEEDS, UP * 3], F32, kind="ExternalInput").ap()
    out_d = nc.dram_tensor("out_acc", [P, 2 * NT], F32, kind="ExternalOutput").ap()

    with TileContext(nc) as tc:
        with tc.tile_pool(name="persist", bufs=1) as pp:
            # ---------- prep: homogeneous lhsT (seeds) and rhs (clouds) ----------
            # rows: [ -1 | 2*seed_x,y,z ] x [ |x|^2 | x,y,z ]
            rhs4 = {}
            xsq = {}
            for cl in ("t", "s"):
                r = pp.tile([4, N], F32, tag="rhs" + cl)
                nc.sync.dma_start(r[1:4, :], xT_d[cl][:])
                x0 = pp.tile([3, N], F32, tag="x0" + cl)
                nc.sync.dma_start(x0[:], xT_d[cl][:])
                rhs4[cl] = r
                xsq[cl] = x0
            lhsT4 = pp.tile([4, 2 * SEEDS], F32)
            nc.sync.dma_start(lhsT4[1:4, :], seedT_d[:])
            wrm = pp.tile([1, 512], F32)
            nc.gpsimd.memset(wrm[:], 1.0)
            with tc.tile_pool(name="prep_ps", bufs=1, space="PSUM") as pps:
                # PE warm-up on junk data (pstate ramp)
                wps = pps.tile([128, 512], F32)
                for _ in range(4):
                    nc.tensor.matmul(wps[:], wrm[:, 0:P], wrm[:], start=True, stop=True)
                for cl in ("t", "s"):
                    r = rhs4[cl]
                    sqx = pp.tile([3, N], F32, tag="sqx" + cl)
                    if cl == "t":
                        nc.scalar.activation(sqx[:], xsq[cl][:], AF.Square)
                    else:
                        nc.vector.tensor_tensor(out=sqx[:], in0=xsq[cl][:], in1=xsq[cl][:],
                                                op=ALU.mult)
                    par = pp.tile([3, N], F32, tag="par" + cl)
                    nc.gpsimd.partition_all_reduce(par[:], sqx[:], channels=3,
                                                   reduce_op=bass_isa.ReduceOp.add)
                    if cl == "t":
                        nc.scalar.copy(r[0:1, :], par[0:1, :])
                    else:
                        nc.vector.tensor_copy(r[0:1, :], par[0:1, :])
            nc.vector.tensor_scalar(out=lhsT4[0:1, :], in0=wrm[:, 0:2 * SEEDS], scalar1=0.0,
                                    scalar2=-1.0, op0=ALU.mult, op1=ALU.add)

            acc = pp.tile([P, 2 * NT], F32)
            epsn = pp.tile([P, 1], F32)
            nc.vector.tensor_scalar(out=epsn[:], in0=acc[:, 0:1], scalar1=0.0,
                                    scalar2=EPS_N, op0=ALU.mult, op1=ALU.add)

            # ---------- main loop ----------
            with tc.tile_pool(name="mn", bufs=4) as mn, \
                 tc.tile_pool(name="st2", bufs=3) as st2, \
                 tc.tile_pool(name="res", bufs=1) as res:

                results = []
                mps = tc.alloc_tile_pool(name="ps_main", bufs=2, space="PSUM")
                for t in ([0, NT, 1, NT + 1] if NT == 2 else list(range(2 * NT))):
                    is_tgt = t < NT
                    col0 = t * P
                    grad = {}
                    udf = {}

                    if is_tgt:
                        # jittered queries of this seed tile: qrm[s, u, c]
                        noise_t = st2.tile([P, UP, 3], F32, tag="noise")
                        nc.sync.dma_start(
                            noise_t[:].rearrange("p u c -> p (u c)"),
                            my_noise_d[t * P:(t + 1) * P, :])
                        seed_t = st2.tile([P, 3], F32, tag="seed")
                        nc.sync.dma_start(seed_t[:], my_tgt_d[t * P:(t + 1) * P, :])
                        qrm = st2.tile([P, UP, 3], F32, tag="qrm")
                        nc.vector.tensor_scalar_mul(qrm[:], noise_t[:], STD)
                        nc.vector.tensor_tensor(
                            out=qrm[:], in0=qrm[:],
                            in1=seed_t[:, None, :].broadcast_to([P, UP, 3]),
                            op=ALU.add)
                    else:
                        qsrc = st2.tile([P, 3], F32, tag="qsrc")
                        nc.sync.dma_start(qsrc[:], my_src_d[(t - NT) * P:(t - NT + 1) * P, :])
                        qsq = st2.tile([P, 3], F32, tag="qsq")
                        nc.vector.tensor_tensor(out=qsq[:], in0=qsrc[:], in1=qsrc[:], op=ALU.mult)
                        q2 = st2.tile([P, 1], F32, tag="q2")
                        nc.vector.tensor_reduce(q2[:], qsq[:], axis=AX.X, op=ALU.add)

                    for cl in (("t", "s") if is_tgt else ("t",)):
                        H = N // 2
                        if is_tgt:
                            t8 = mn.tile([P, 2, 8], F32, tag="t8")
                            ix = mn.tile([P, 2, 8], U32, tag="ix")
                            for h in range(2):
                                ph = mps.tile([P, H], F32, tag="ph%d" % h)
                                for j in range(0, H, 512):
                                    i = h * H + j
                                    nc.tensor.matmul(ph[:, j:j + 512],
                                                     lhsT4[:, col0:col0 + P],
                                                     rhs4[cl][:, i:i + 512],
                                                     start=True, stop=True)
                                nc.vector.max(t8[:, h, :], ph[:])
                                nc.vector.max_index(ix[:, h, :], t8[:, h, :], ph[:])
                            xc = mn.tile([P, 2 * CH, 3], F32, tag="xc")
                            for h in range(2):
                                for k in range(CH):
                                    nc.gpsimd.indirect_dma_start(
                                        out=xc[:, h * CH + k, :],
                                        out_offset=None,
                                        in_=pts_d[cl][:],
                                        in_offset=bass.IndirectOffsetOnAxis(
                                            ap=ix[:, h, k:k + 1], axis=0),
                                        element_offset=h * H * 3,
                                    )
                            # ---- stage 2: refine jitters against 16 candidates ----
                            dx = st2.tile([P, UP, 2 * CH, 3], F32, tag="dx")
                            nc.vector.tensor_tensor(
                                out=dx[:],
                                in0=qrm[:, :, None, :].broadcast_to([P, UP, 2 * CH, 3]),
                                in1=xc[:, None, :, :].broadcast_to([P, UP, 2 * CH, 3]),
                                op=ALU.subtract)
                            sqd = st2.tile([P, UP, 2 * CH, 3], F32, tag="sqd")
                            nc.scalar.activation(sqd[:], dx[:], AF.Square)
                            nd2 = st2.tile([P, UP, 2 * CH], F32, tag="nd2")
                            nc.vector.tensor_reduce(nd2[:], sqd[:], axis=AX.X, op=ALU.add,
                                                    negate=True)
                            t8u = st2.tile([P, UP, 8], F32, tag="t8u")
                            for u in range(UP):
                                nc.vector.max(t8u[:, u, :], nd2[:, u, :])
                            mask = st2.tile([P, UP, 2 * CH], F32, tag="mask")
                            nc.vector.tensor_tensor(
                                out=mask[:], in0=nd2[:],
                                in1=t8u[:, :, 4:5].broadcast_to([P, UP, 2 * CH]),
                                op=ALU.is_ge)
                            dpe = st2.tile([P, UP, 2 * CH], F32, tag="dpe")
                            nc.vector.tensor_scalar(
                                out=dpe[:], in0=nd2[:], scalar1=-1.0, scalar2=EPS_D,
                                op0=ALU.mult, op1=ALU.add)
                            rd = st2.tile([P, UP, 2 * CH], F32, tag="rd")
                            nc.vector.reciprocal(rd[:], dpe[:])
                            w = st2.tile([P, UP, 2 * CH], F32, tag="w")
                            nc.vector.tensor_tensor(out=w[:], in0=mask[:], in1=rd[:], op=ALU.mult)
                            sinv = st2.tile([P, UP], F32, tag="sinv")
                            nc.vector.tensor_reduce(sinv[:], w[:], axis=AX.X, op=ALU.add)
                            wx = st2.tile([P, UP, 2 * CH, 3], F32, tag="wx")
                            nc.vector.tensor_tensor(
                                out=wx[:],
                                in0=w[:, :, :, None].broadcast_to([P, UP, 2 * CH, 3]),
                                in1=xc[:, None, :, :].broadcast_to([P, UP, 2 * CH, 3]),
                                op=ALU.mult)
                            S = st2.tile([P, UP, 3], F32, tag="S")
                            nc.vector.tensor_reduce(
                                S[:], wx[:].rearrange("p u k c -> p u c k"),
                                axis=AX.X, op=ALU.add)
                            rsv = st2.tile([P, UP], F32, tag="rsv")
                            nc.vector.reciprocal(rsv[:], sinv[:])
                            g = res.tile([P, UP, 3], F32, tag="g%d%s" % (t, cl))
                            nc.vector.tensor_tensor(
                                out=g[:], in0=S[:],
                                in1=rsv[:, :, None].broadcast_to([P, UP, 3]),
                                op=ALU.mult)
                            nc.vector.tensor_tensor(out=g[:], in0=qrm[:], in1=g[:], op=ALU.subtract)
                            # udf = || g + eps ||
                            gp = st2.tile([P, UP, 3], F32, tag="gp")
                            nc.scalar.activation(gp[:], g[:], AF.Square, bias=epsn[:, 0:1])
                            ss = res.tile([P, UP], F32, tag="ss%d%s" % (t, cl))
                            nc.vector.tensor_reduce(ss[:], gp[:], axis=AX.X, op=ALU.add)
                            grad[cl] = g
                            udf[cl] = ss
                        else:
                            # ---- src queries: exact top-5 from sorted top-8 ----
                            nd = mn.tile([P, N], F32, tag="nds")
                            for h in range(2):
                                ph = mps.tile([P, H], F32, tag="ph%d" % h)
                                for j in range(0, H, 512):
                                    i = h * H + j
                                    nc.tensor.matmul(ph[:, j:j + 512],
                                                     lhsT4[:, col0:col0 + P],
                                                     rhs4[cl][:, i:i + 512],
                                                     start=True, stop=True)
                                nc.scalar.copy(nd[:, h * H:(h + 1) * H], ph[:])
                            t8 = mn.tile([P, 8], F32, tag="t8s")
                            ix = mn.tile([P, 8], U32, tag="ixs")
                            nc.vector.max(t8[:], nd[:])
                            nc.vector.max_index(ix[:], t8[:], nd[:])
                            xc5 = mn.tile([P, 5, 3], F32, tag="xc5")
                            for k in range(5):
                                nc.gpsimd.indirect_dma_start(
                                    out=xc5[:, k, :],
                                    out_offset=None,
                                    in_=pts_d[cl][:],
                                    in_offset=bass.IndirectOffsetOnAxis(
                                        ap=ix[:, k:k + 1], axis=0),
                                )
                            d5 = st2.tile([P, 5], F32, tag="d5")
                            # d = |q|^2 - v, clamped at 0
                            nc.vector.tensor_scalar(out=d5[:], in0=t8[:, 0:5], scalar1=-1.0,
                                                    scalar2=q2[:], op0=ALU.mult, op1=ALU.add)
                            nc.vector.tensor_scalar_max(d5[:], d5[:], 0.0)
                            inv5 = st2.tile([P, 5], F32, tag="inv5")
                            nc.vector.tensor_scalar_add(inv5[:], d5[:], EPS_D)
                            nc.vector.reciprocal(inv5[:], inv5[:])
                            sinv = st2.tile([P, 1], F32, tag="sinvs")
                            nc.vector.tensor_reduce(sinv[:], inv5[:], axis=AX.X, op=ALU.add)
                            wx = st2.tile([P, 5, 3], F32, tag="wxs")
                            nc.vector.tensor_tensor(
                                out=wx[:],
                                in0=inv5[:, :, None].broadcast_to([P, 5, 3]),
                                in1=xc5[:], op=ALU.mult)
                            S = st2.tile([P, 3], F32, tag="Ss")
                            nc.vector.tensor_reduce(
                                S[:], wx[:].rearrange("p k c -> p c k"),
                                axis=AX.X, op=ALU.add)
                            rsv = st2.tile([P, 1], F32, tag="rsvs")
                            nc.vector.reciprocal(rsv[:], sinv[:])
                            g = res.tile([P, 3], F32, tag="g%d%s" % (t, cl))
                            nc.vector.tensor_scalar(
                                out=g[:], in0=S[:], scalar1=rsv[:], scalar2=None,
                                op0=ALU.mult)
                            nc.vector.tensor_tensor(out=g[:], in0=qsrc[:], in1=g[:], op=ALU.subtract)
                            gp = st2.tile([P, 3], F32, tag="gps")
                            nc.scalar.activation(gp[:], g[:], AF.Square, bias=epsn[:, 0:1])
                            ss = res.tile([P, 1], F32, tag="ss%d%s" % (t, cl))
                            nc.vector.tensor_reduce(ss[:], gp[:], axis=AX.X, op=ALU.add)
                            grad[cl] = g
                            udf[cl] = ss

                    results.append((t, is_tgt, dict(grad), dict(udf)))
                mps.release()

                # ---- combine per tile (sqrts batched within tile) ----
                uds = {}
                etiles = {}
                for t, is_tgt, grad, udf in results:
                    M = UP if is_tgt else 1
                    for cl in (("t", "s") if is_tgt else ("t",)):
                        ud = res.tile([P, M], F32, tag="ud%d%s" % (t, cl))
                        nc.scalar.activation(ud[:], udf[cl][:], AF.Sqrt)
                        uds[(t, cl)] = ud
                    M = UP if is_tgt else 1
                    gerr = st2.tile([P, M], F32, tag="gerr")
                    if is_tgt:
                        gd = st2.tile([P, M, 3], F32, tag="gd")
                        nc.vector.tensor_tensor(out=gd[:], in0=grad["s"][:], in1=grad["t"][:],
                                                op=ALU.subtract)
                        nc.vector.tensor_reduce(gerr[:], gd[:], axis=AX.X, op=ALU.add,
                                                apply_absolute_value=True)
                        ue = st2.tile([P, M], F32, tag="ue")
                        nc.vector.tensor_tensor(out=ue[:], in0=uds[(t, "t")][:],
                                                in1=uds[(t, "s")][:], op=ALU.subtract)
                        ua = st2.tile([P, M], F32, tag="ua")
                        nc.vector.tensor_reduce(ua[:], ue[:, :, None], axis=AX.X, op=ALU.add,
                                                apply_absolute_value=True)
                    else:
                        # src-vs-src is a self-match: udf_s ~ 0, grad_s ~ 0
                        nc.vector.tensor_reduce(gerr[:], grad["t"][:, None, :], axis=AX.X,
                                                op=ALU.add, apply_absolute_value=True)
                        ua = uds[(t, "t")]
                    e = res.tile([P, M], F32, tag="e%d" % t)
                    nc.vector.tensor_tensor(out=e[:], in0=ua[:], in1=gerr[:], op=ALU.add)
                    wexp = st2.tile([P, M], F32, tag="wexp")
                    nc.scalar.activation(wexp[:], e[:], AF.Exp, scale=-BETA)
                    term = st2.tile([P, M], F32, tag="term")
                    nc.vector.tensor_tensor(out=term[:], in0=e[:], in1=wexp[:], op=ALU.mult)
                    if M > 1:
                        nc.vector.tensor_reduce(acc[:, t:t + 1], term[:], axis=AX.X, op=ALU.add)
                    else:
                        nc.vector.tensor_copy(acc[:, t:t + 1], term[:])

            nc.sync.dma_start(out_d[:], acc[:])

    nc.compile()
    return nc


def _get_prog():
    global _PROG
    if _PROG is None:
        _PROG = _build()
    return _PROG


def kernel(src, tgt, noise):
    from concourse.bass_utils import run_bass_kernel_spmd

    src = np.ascontiguousarray(np.asarray(src, dtype=np.float32).reshape(N, 3))
    tgt = np.ascontiguousarray(np.asarray(tgt, dtype=np.float32).reshape(N, 3))
    noise = np.ascontiguousarray(np.asarray(noise, dtype=np.float32).reshape(N, UP, 3))

    nc = _get_prog()

    tgtT = np.ascontiguousarray(tgt.T)
    srcT = np.ascontiguousarray(src.T)
    in_maps = []
    for c in range(NCORE):
        sl = slice(c * SEEDS, (c + 1) * SEEDS)
        in_maps.append({
            "tgt_pts": tgt,
            "src_pts": src,
            "tgtT": tgtT,
            "srcT": srcT,
            "seedT": np.ascontiguousarray(
                2.0 * np.concatenate([tgt[sl].T, src[sl].T], axis=1)),
            "my_tgt": np.ascontiguousarray(tgt[sl]),
            "my_src": np.ascontiguousarray(src[sl]),
            "my_noise": np.ascontiguousarray(noise[sl].reshape(SEEDS, UP * 3)),
        })

    trace = os.environ.get("KNN_TRACE", "") == "1"
    res = run_bass_kernel_spmd(nc, in_maps, list(range(NCORE)), trace=trace)
    global LAST_EXEC_NS
    LAST_EXEC_NS = res.exec_time_ns

    total = np.float64(0.0)
    for c in range(NCORE):
        total += res.results[c]["out_acc"].astype(np.float64).sum()
    return np.asarray(np.float32(total) / 1.0 / NQ, dtype=np.float32)


if __name__ == "__main__":
    # numpy self-check
    rng = np.random.default_rng(0)
    src = rng.standard_normal((1, N, 3)).astype(np.float32)
    tgt = rng.standard_normal((1, N, 3)).astype(np.float32)
    noise = rng.standard_normal((1, N, UP, 3)).astype(np.float32)

    def udf_np(x, q):
        d2 = ((q[:, None, :] - x[None, :, :]) ** 2).sum(-1)
        idx = np.argpartition(d2, 5, axis=1)[:, :5]
        dk = np.maximum(np.take_along_axis(d2, idx, 1), 0)
        inv = 1.0 / (dk + EPS_D)
        wk = inv / inv.sum(1, keepdims=True)
        g = ((q[:, None, :] - x[idx]) * wk[..., None]).sum(1)
        u = np.sqrt(((g + EPS_N) ** 2).sum(-1))
        return u, g

    q = np.concatenate([(tgt[0][:, None, :] + noise[0] * STD).reshape(-1, 3), src[0]], 0)
    ut, gt = udf_np(tgt[0], q)
    us, gs = udf_np(src[0], q)
    err = np.abs(ut - us)
    gerr = np.abs(gs - gt).sum(-1)
    wq = np.exp(-(err + gerr) * BETA)
    expected = ((err + gerr) * wq).sum() / q.shape[0]

    got = kernel(src=src, tgt=tgt, noise=noise)
    print("expected:", expected)
    print("got     :", got)
    print("rel err :", abs(got - expected) / abs(expected))
    print("exec_ns :", LAST_EXEC_NS)
